# revision 2
# baseline (speedup 1.0000x reference)
"""Trainium2 Bass kernel for quantized dense layer with Hadamard rotations.

Math (see reference): y = (H2 @ (sq(H2@x) @ sq(w@H1)) @ H1)/(64*64) + bias,
where sq() is per-tensor symmetric int8 stochastic quantization.

Structure exploited: Sylvester Hadamards factor as Kronecker products
(H4096 = H32 (x) H128).  Every Hadamard application is a per-128-tile PE
matmul against an H128 constant plus a cross-tile DVE add/sub butterfly.
The core GEMM runs on int8-valued operands stored as bf16 (exact:
|acc| < 2^25) at full PE rate.  Stochastic rounding is computed as
rint(x*scale + (0.5 - noise)) via the fp32->int32 round-to-nearest cast,
with (0.5 - noise) precomputed on the host.

Sharding (8 cores): the IN axis is split 8 ways for forward transforms +
quantization (column/row-local).  Global quant scales via two 1-scalar
AllReduce-max ops (split so x-quant + AllGather overlap the w transform).
Quantized operands are PE-transposed into collective-friendly layouts,
exchanged via AllGather (activations) and AllToAll (weights).  Each core
computes yr[:, Fk] and applies every inverse-transform factor except the
outer H8 on features, which is folded into the host-side unshard (an 8x8
combine over gathered per-core outputs).
"""
import sys, os
sys.path.insert(0, '/opt/trn_rl_repo')
import numpy as np

B, IN, F = 4096, 2048, 4096
NCORES = 8
CS = IN // NCORES      # 256  per-core IN slice
FS = F // NCORES       # 512  per-core feature block
BT = B // 128          # 32   batch tiles
KT = IN // 128         # 16   contraction tiles
FT = FS // 128         # 4    feature tiles in a block
QMAX = 127.0
KSTOP = int(os.environ.get("KSTOP", "8"))

_cache = {}


class _StopBuild(Exception):
    pass


def _sylvester(n):
    h = np.array([[1.0]], dtype=np.float32)
    while h.shape[0] < n:
        h = np.block([[h, h], [h, -h]])
    return h


def _build():
    from concourse import bass, bacc, tile, mybir
    import concourse.bass_isa as bass_isa

    DT = mybir.dt.float32
    BF = mybir.dt.bfloat16
    I32 = mybir.dt.int32
    A = mybir.AluOpType
    npbf = mybir.dt.np(BF)

    nc = bacc.Bacc("TRN2", target_bir_lowering=False, debug=False,
                   num_devices=NCORES)

    xk = nc.dram_tensor("xk", [B, CS], DT, kind="ExternalInput")
    nk = nc.dram_tensor("nk", [B, CS], DT, kind="ExternalInput")   # 0.5-noise_x
    wk = nc.dram_tensor("wk", [F, CS], DT, kind="ExternalInput")   # w slice^T
    mk = nc.dram_tensor("mk", [F, CS], DT, kind="ExternalInput")   # (.5-noise_w)^T
    out = nc.dram_tensor("out", [FS, B], DT, kind="ExternalOutput")

    sx_i = nc.dram_tensor("sx_i", [1, 1], DT)
    sx_o = nc.dram_tensor("sx_o", [1, 1], DT, addr_space="Shared")
    sw_i = nc.dram_tensor("sw_i", [1, 1], DT)
    sw_o = nc.dram_tensor("sw_o", [1, 1], DT, addr_space="Shared")
    xqc = nc.dram_tensor("xqc", [CS, B], BF)                       # xq^T slice
    xqg = nc.dram_tensor("xqg", [IN, B], BF, addr_space="Shared")  # full xq^T
    wac = nc.dram_tensor("wac", [IN, FS], BF)                      # A2A contrib
    wblk = nc.dram_tensor("wblk", [IN, FS], BF)                    # wq[:, Fk]

    h128f_d = nc.inline_tensor(_sylvester(128), name="h128f")
    h128b_d = nc.inline_tensor(_sylvester(128).astype(npbf), name="h128b")
    idb_d = nc.inline_tensor(np.eye(128, dtype=np.float32).astype(npbf),
                             name="idb")
    rg = [list(range(NCORES))]

    NB = 32 * CS  # 8192 free columns in a fwd big tile

    def butterfly(nc, bufs, T, blk0, A):
        """FWHT across the tile-index axis of big tensors [128, T*blk0].
        Emitted as 2D contiguous ops (per hi-group) for DVE fast modes."""
        n = T.bit_length() - 1
        for s in range(n):
            cur, nxt = bufs(s)
            blk = blk0 << s
            hi = T >> (s + 1)
            for h in range(hi):
                a0 = h * 2 * blk
                a1 = a0 + blk
                nc.vector.tensor_tensor(nxt[:, a0:a0 + blk],
                                        cur[:, a0:a0 + blk],
                                        cur[:, a1:a1 + blk], op=A.add)
                nc.vector.tensor_tensor(nxt[:, a1:a1 + blk],
                                        cur[:, a0:a0 + blk],
                                        cur[:, a1:a1 + blk], op=A.subtract)

    with tile.TileContext(nc) as tc:
      try:
        with tc.tile_pool(name="consts", bufs=1) as cpool:
            h128f = cpool.tile([128, 128], DT)
            h128b = cpool.tile([128, 128], BF)
            idb = cpool.tile([128, 128], BF)
            nc.sync.dma_start(h128f[:], h128f_d[:])
            nc.sync.dma_start(h128b[:], h128b_d[:])
            nc.sync.dma_start(idb[:], idb_d[:])

            # ================= forward transforms + quant =================
            with tc.tile_pool(name="fwd", bufs=2) as fp_, \
                 tc.tile_pool(name="fin", bufs=4) as fin, \
                 tc.tile_pool(name="fps", bufs=1, space="PSUM") as fps, \
                 tc.tile_pool(name="qtmp", bufs=2) as qtmp, \
                 tc.tile_pool(name="qT", bufs=4) as qTp, \
                 tc.tile_pool(name="qsc", bufs=1) as qsc:

                def fwd_side(src_tile_ap, ntiles, side):
                    bigA = fp_.tile([128, NB], DT, tag="bigA",
                                    name=f"bigA{side}")
                    bigB = fp_.tile([128, NB], DT, tag="bigB",
                                    name=f"bigB{side}")
                    for o in range(ntiles):
                        t = fin.tile([128, CS], DT, tag="fin", name="fint")
                        nc.sync.dma_start(t[:], src_tile_ap(o))
                        ps = fps.tile([128, CS], DT, tag="ps", name="fpst",
                                      bufs=4)
                        nc.tensor.matmul(ps[:], h128f[:], t[:], start=True,
                                         stop=True)
                        nc.vector.tensor_copy(bigA[:, o * CS:(o + 1) * CS],
                                              ps[:])
                    butterfly(nc, lambda s: (bigA, bigB) if s % 2 == 0
                              else (bigB, bigA), 32, CS, A)
                    return bigB  # 5 stages -> result in B

                def scale_trigger(big, tag, cc_in, cc_out):
                    am = qsc.tile([128, 1], DT, tag=f"am{tag}",
                                  name=f"am{tag}")
                    nc.vector.tensor_reduce(am[:], big[:],
                                            axis=mybir.AxisListType.X,
                                            op=A.max,
                                            apply_absolute_value=True)
                    red = qsc.tile([128, 1], DT, tag=f"rd{tag}",
                                   name=f"rd{tag}")
                    nc.gpsimd.partition_all_reduce(
                        red[:], am[:], channels=128,
                        reduce_op=bass_isa.ReduceOp.absmax)
                    nc.sync.dma_start(cc_in[:], red[0:1, 0:1])
                    nc.gpsimd.collective_compute(
                        "AllReduce", A.max, replica_groups=rg,
                        ins=[cc_in.ap().opt()], outs=[cc_out.ap().opt()])

                def scale_finish(tag, cc_out):
                    sg = qsc.tile([1, 1], DT, tag=f"sg{tag}",
                                  name=f"sg{tag}")
                    nc.sync.dma_start(sg[0:1, :], cc_out[:])
                    # r = QMAX/s with one newton step
                    r0 = qsc.tile([1, 1], DT, tag=f"r0{tag}", name=f"r0{tag}")
                    nc.vector.reciprocal(r0[0:1, :], sg[0:1, :])
                    mr = qsc.tile([1, 1], DT, tag=f"mr{tag}", name=f"mr{tag}")
                    nc.vector.tensor_tensor(mr[0:1, :], sg[0:1, :],
                                            r0[0:1, :], op=A.mult)
                    tw = qsc.tile([1, 1], DT, tag=f"tw{tag}", name=f"tw{tag}")
                    nc.vector.tensor_scalar(tw[0:1, :], mr[0:1, :], -1.0, 2.0,
                                            op0=A.mult, op1=A.add)
                    r1 = qsc.tile([1, 1], DT, tag=f"r1{tag}", name=f"r1{tag}")
                    nc.vector.tensor_tensor(r1[0:1, :], r0[0:1, :],
                                            tw[0:1, :], op=A.mult)
                    r127 = qsc.tile([1, 1], DT, tag=f"rq{tag}",
                                    name=f"rq{tag}")
                    nc.vector.tensor_scalar_mul(r127[0:1, :], r1[0:1, :],
                                                QMAX)
                    rb = qsc.tile([128, 1], DT, tag=f"rb{tag}",
                                  name=f"rb{tag}")
                    nc.gpsimd.partition_broadcast(rb[:, 0:1], r127[0:1, 0:1])
                    return sg, rb

                CH = 1024   # quant chunk = 4 o-tiles

                def quant_transpose(big, rb, noise_ap, side, tiles_T):
                    """quantize [128, NB] -> int-valued bf16, PE-transpose
                    128-blocks into tiles_T[h][128, B] (h = col-half)."""
                    nt_ch = CH // CS  # 4
                    for ch in range(NB // CH):
                        nz = qtmp.tile([128, CH], DT, tag="nz", name="nzt")
                        nc.sync.dma_start(
                            nz[:].rearrange("p (o c) -> p o c", o=nt_ch),
                            noise_ap(ch))
                        qi = qtmp.tile([128, CH], I32, tag="qi", name="qit")
                        nc.vector.scalar_tensor_tensor(
                            qi[:], big[:, ch * CH:(ch + 1) * CH], rb[:, 0:1],
                            nz[:], op0=A.mult, op1=A.add)
                        qb = qtmp.tile([128, CH], BF, tag=f"qb{side}",
                                       name="qbt")
                        nc.vector.tensor_copy(qb[:], qi[:])
                        for ol in range(nt_ch):
                            o = ch * nt_ch + ol
                            for h in range(2):
                                ps = fps.tile([128, 128], BF, tag="tps",
                                              name="tpst", bufs=4)
                                nc.tensor.transpose(
                                    ps[:],
                                    qb[:, ol * CS + h * 128:
                                       ol * CS + (h + 1) * 128], idb[:])
                                eng = nc.scalar if (o + h) % 2 else nc.vector
                                if eng is nc.scalar:
                                    nc.scalar.copy(
                                        tiles_T[h][:, o * 128:(o + 1) * 128],
                                        ps[:])
                                else:
                                    nc.vector.tensor_copy(
                                        tiles_T[h][:, o * 128:(o + 1) * 128],
                                        ps[:])

                # ---- x side ----
                xrB = fwd_side(lambda o: xk[o * 128:(o + 1) * 128, :], BT,
                               "x")
                scale_trigger(xrB, "x", sx_i, sx_o)
                sgx, rbx = scale_finish("x", sx_o)
                xT = [qTp.tile([128, B], BF, tag="qT", name=f"xT{h}")
                      for h in range(2)]
                quant_transpose(
                    xrB, rbx,
                    lambda c: nk[c * 512:(c + 1) * 512, :]
                    .rearrange("(o p) c -> p o c", p=128), "x", xT)
                for h in range(2):
                    nc.sync.dma_start(xqc[h * 128:(h + 1) * 128, :],
                                      xT[h][:])

                # ---- w side ----
                wrB = fwd_side(lambda o: wk[o * 128:(o + 1) * 128, :],
                               F // 128, "w")
                # AR-w first on the collective queue, AG right behind it
                scale_trigger(wrB, "w", sw_i, sw_o)
                if KSTOP >= 5:
                    nc.gpsimd.collective_compute(
                        "AllGather", A.bypass, replica_groups=rg,
                        ins=[xqc.ap().opt()], outs=[xqg.ap().opt()])
                sgw, rbw = scale_finish("w", sw_o)
                wT = [qTp.tile([128, B], BF, tag="qT", name=f"wT{h}")
                      for h in range(2)]
                quant_transpose(
                    wrB, rbw,
                    lambda c: mk[c * 512:(c + 1) * 512, :]
                    .rearrange("(o p) r -> p o r", p=128), "w", wT)
                for a in range(NCORES):
                    for h in range(2):
                        nc.sync.dma_start(
                            wac[a * CS + h * 128:a * CS + (h + 1) * 128, :],
                            wT[h][:, a * FS:(a + 1) * FS])

                # alpha = sx*sw/(QMAX^2 * 2^24)  (before the A2A trigger)
                al = qsc.tile([1, 1], DT, tag="al", name="al")
                nc.vector.tensor_tensor(al[0:1, 0:1], sgx[0:1, 0:1],
                                        sgw[0:1, 0:1], op=A.mult)
                nc.vector.tensor_scalar_mul(
                    al[0:1, 0:1], al[0:1, 0:1],
                    float(1.0 / (QMAX * QMAX * (1 << 24))))
                alb = qsc.tile([128, 1], DT, tag="alb", name="alb")
                nc.gpsimd.partition_broadcast(alb[:, 0:1], al[0:1, 0:1])
                if KSTOP >= 5:
                    nc.gpsimd.collective_compute(
                        "AllToAll", A.bypass, replica_groups=rg,
                        ins=[wac.ap().opt()], outs=[wblk.ap().opt()])

            if KSTOP < 6:
                raise _StopBuild()

            # ================= GEMM + inverse transforms =================
            with tc.tile_pool(name="yrp", bufs=1) as yrp, \
                 tc.tile_pool(name="gps", bufs=1, space="PSUM") as gps:
                yrb = yrp.tile([128, BT * FS], BF, tag="yrb", name="yrb")
                with tc.tile_pool(name="gem", bufs=KT) as gem:
                    xs, ws = [], []
                    for kt in range(KT):
                        tx = gem.tile([128, B], BF, tag="xs", name="xst")
                        nc.sync.dma_start(tx[:],
                                          xqg[kt * 128:(kt + 1) * 128, :])
                        xs.append(tx)
                        tw_ = gem.tile([128, FS], BF, tag="ws", name="wst")
                        nc.sync.dma_start(tw_[:],
                                          wblk[kt * 128:(kt + 1) * 128, :])
                        ws.append(tw_)
                    # kt-outer within groups of 8 batch tiles: GEMM starts
                    # as soon as the first k chunks land
                    for g in range(BT // 8):
                        pss = [gps.tile([128, FS], DT, tag="gp",
                                        name=f"gpt{g}_{i}", bufs=8)
                               for i in range(8)]
                        for kt in range(KT):
                            for i in range(8):
                                bo = g * 8 + i
                                nc.tensor.matmul(
                                    pss[i][:],
                                    xs[kt][:, bo * 128:(bo + 1) * 128],
                                    ws[kt][:], start=(kt == 0),
                                    stop=(kt == KT - 1))
                        for i in range(8):
                            bo = g * 8 + i
                            nc.vector.tensor_scalar(
                                yrb[:, bo * FS:(bo + 1) * FS], pss[i][:],
                                alb[:, 0:1], None, op0=A.mult)

                if KSTOP < 7:
                    raise _StopBuild()

                with tc.tile_pool(name="inv2", bufs=1) as invp:
                    # batch inverse: H128 per tile (in-place) + H32 butterfly
                    uB = invp.tile([128, BT * FS], BF, tag="gb", name="uB",
                                   bufs=2)
                    for bo in range(BT):
                        ps = gps.tile([128, FS], DT, tag="gp", name="gpt2",
                                      bufs=8)
                        nc.tensor.matmul(ps[:], h128b[:],
                                         yrb[:, bo * FS:(bo + 1) * FS],
                                         start=True, stop=True)
                        nc.vector.tensor_copy(yrb[:, bo * FS:(bo + 1) * FS],
                                              ps[:])
                    butterfly(nc, lambda s: (yrb, uB) if s % 2 == 0
                              else (uB, yrb), 32, FS, A)
                    # result in uB

                    # transpose [b-part, f] -> [f-part, b]
                    uT = [invp.tile([128, B], BF, tag="uT", name=f"uT{i}",
                                    bufs=4) for i in range(FT)]
                    for bo in range(BT):
                        for ft in range(FT):
                            ps = gps.tile([128, 128], BF, tag="gp",
                                          name="tpt", bufs=8)
                            nc.tensor.transpose(
                                ps[:],
                                uB[:, bo * FS + ft * 128:
                                   bo * FS + (ft + 1) * 128], idb[:])
                            nc.vector.tensor_copy(
                                uT[ft][:, bo * 128:(bo + 1) * 128], ps[:])

                    if KSTOP < 8:
                        raise _StopBuild()

                    # feature inverse: H128 per f-tile + H4 butterfly
                    zb = invp.tile([128, FT * B], BF, tag="gb", name="zb",
                                   bufs=2)
                    for ft in range(FT):
                        for nb in range(B // 512):
                            ps = gps.tile([128, 512], DT, tag="gp",
                                          name="zpt", bufs=8)
                            nc.tensor.matmul(
                                ps[:], h128b[:],
                                uT[ft][:, nb * 512:(nb + 1) * 512],
                                start=True, stop=True)
                            nc.scalar.copy(
                                zb[:, ft * B + nb * 512:
                                   ft * B + (nb + 1) * 512], ps[:])
                    z2 = invp.tile([128, FT * B], BF, tag="gb", name="z2",
                                   bufs=2)
                    # H4 stage 0: pairs (0,1),(2,3)
                    for h in range(2):
                        a0, a1 = h * 2 * B, h * 2 * B + B
                        nc.vector.tensor_tensor(z2[:, a0:a0 + B],
                                                zb[:, a0:a0 + B],
                                                zb[:, a1:a1 + B], op=A.add)
                        nc.vector.tensor_tensor(z2[:, a1:a1 + B],
                                                zb[:, a0:a0 + B],
                                                zb[:, a1:a1 + B],
                                                op=A.subtract)
                    # H4 stage 1 (pairs (0,2),(1,3)): fp32 chunks + DMA out
                    CB = 2048
                    for t in range(2):
                        for ft_o, sgn in ((t, A.add), (t + 2, A.subtract)):
                            for cb in range(B // CB):
                                vch = invp.tile([128, CB], DT, tag="vch",
                                                name="vch", bufs=4)
                                nc.vector.tensor_tensor(
                                    vch[:],
                                    z2[:, t * B + cb * CB:
                                       t * B + (cb + 1) * CB],
                                    z2[:, (t + 2) * B + cb * CB:
                                       (t + 2) * B + (cb + 1) * CB],
                                    op=sgn)
                                nc.sync.dma_start(
                                    out[ft_o * 128:(ft_o + 1) * 128,
                                        cb * CB:(cb + 1) * CB], vch[:])
      except _StopBuild:
        pass
    nc.compile()
    return nc


def make_in_maps(inputs):
    x = np.asarray(inputs["inputs"], np.float32)
    w = np.asarray(inputs["kernel"], np.float32)
    nxp = 0.5 - np.asarray(inputs["noise_x"], np.float32)
    nwp = 0.5 - np.asarray(inputs["noise_w"], np.float32)

    in_maps = []
    for k in range(NCORES):
        cs = slice(k * CS, (k + 1) * CS)
        in_maps.append({
            "xk": np.ascontiguousarray(x[:, cs]),
            "nk": np.ascontiguousarray(nxp[:, cs]),
            "wk": np.ascontiguousarray(w[cs, :].T),
            "mk": np.ascontiguousarray(nwp[cs, :].T),
        })
    return in_maps


def kernel(**inputs):
    from concourse.bass_utils import run_bass_kernel_spmd

    if "nc" not in _cache:
        _cache["nc"] = _build()
    nc = _cache["nc"]

    bias = np.asarray(inputs["bias"], np.float32)
    in_maps = make_in_maps(inputs)

    res = run_bass_kernel_spmd(nc, in_maps, list(range(NCORES)))
    V = np.stack([r["out"] for r in res.results])          # [a', g, b]
    H8 = _sylvester(8)
    yT = (H8 @ V.reshape(NCORES, -1)).reshape(F, B)        # [f, b], f=a*512+g
    y = np.ascontiguousarray(yT.T) + bias[None, :]
    return y.astype(np.float32)



# revision 6
# speedup vs baseline: 1.6192x; 1.6192x over previous
"""Trainium2 Bass kernel for quantized dense layer with Hadamard rotations.

Math (reference): y = (H2 @ (sq(H2@x) @ sq(w@H1)) @ H1)/4096 + bias,
sq() = per-tensor symmetric int8 stochastic quantization.

Sharding (8 cores), per the data-parallel + per-shard-Hadamard hint:
Sylvester Hadamards factor as Kronecker products; the cross-shard H32
factors are folded into the host-side shard/unshard combines, while
each core applies the per-shard H128 factors on the PE (bf16 hi+lo
split of the fp32 operands, accumulated in one PSUM group — exact to
~2^-18), computes the global quant scales via two 1-scalar AllReduces,
quantizes (stochastic rounding via the fp32->int32 round-to-nearest
cast with host-precomputed 0.5-noise), PE-transposes into GEMM layout,
AllGathers the quantized activations (w arrives feature-sharded so no
AllToAll is needed), runs the int-exact bf16 GEMM, and applies the
inverse per-shard H128 factors (fused transpose matmuls, fp16).

Host: cross-shard H32 combines (pre: batch-low bits of x, feature-high
bits of w; post: the mirror factors on the gathered output), layout
prep, hi/lo dtype split, bias.
"""
import sys
sys.path.insert(0, '/opt/trn_rl_repo')
import numpy as np
import ml_dtypes

B, IN, F = 4096, 2048, 4096
NCORES = 8
CS = IN // NCORES      # 256  per-core IN slice of x
FS = F // NCORES       # 512  per-core feature block of w
QMAX = 127.0
BF16 = ml_dtypes.bfloat16
FP16 = np.float16

_cache = {}


def _sylvester(n):
    h = np.array([[1.0]], dtype=np.float32)
    while h.shape[0] < n:
        h = np.block([[h, h], [h, -h]])
    return h


def _build():
    from concourse import bass, bacc, tile, mybir
    import concourse.bass_isa as bass_isa

    DT = mybir.dt.float32
    BF = mybir.dt.bfloat16
    F16 = mybir.dt.float16
    I32 = mybir.dt.int32
    A = mybir.AluOpType
    npbf = mybir.dt.np(BF)
    npf16 = mybir.dt.np(F16)

    nc = bacc.Bacc("TRN2", target_bir_lowering=False, debug=False,
                   num_devices=NCORES)

    # host-prepped inputs
    xhh = nc.dram_tensor("xhh", [128, 8192], BF, kind="ExternalInput")
    xhl = nc.dram_tensor("xhl", [128, 8192], BF, kind="ExternalInput")
    nk = nc.dram_tensor("nk", [128, 8192], F16, kind="ExternalInput")
    whh = nc.dram_tensor("whh", [512, 2048], BF, kind="ExternalInput")
    whl = nc.dram_tensor("whl", [512, 2048], BF, kind="ExternalInput")
    mk = nc.dram_tensor("mk", [512, 2048], F16, kind="ExternalInput")
    out = nc.dram_tensor("out", [512, 4096], F16, kind="ExternalOutput")

    sx_i = nc.dram_tensor("sx_i", [1, 1], DT)
    sx_o = nc.dram_tensor("sx_o", [1, 1], DT, addr_space="Shared")
    sw_i = nc.dram_tensor("sw_i", [1, 1], DT)
    sw_o = nc.dram_tensor("sw_o", [1, 1], DT, addr_space="Shared")
    xqc_a = nc.dram_tensor("xqc_a", [128, 4096], BF)
    xqc_b = nc.dram_tensor("xqc_b", [128, 4096], BF)
    xqg_a = nc.dram_tensor("xqg_a", [1024, 4096], BF, addr_space="Shared")
    xqg_b = nc.dram_tensor("xqg_b", [1024, 4096], BF, addr_space="Shared")

    h128b_d = nc.inline_tensor(_sylvester(128).astype(npbf), name="h128b")
    h128h_d = nc.inline_tensor(_sylvester(128).astype(npf16), name="h128h")
    idb_d = nc.inline_tensor(np.eye(128, dtype=np.float32).astype(npbf),
                             name="idb")
    rg = [list(range(NCORES))]

    with tile.TileContext(nc) as tc:
      with tc.tile_pool(name="consts", bufs=1) as cpool, \
           tc.tile_pool(name="big", bufs=1) as bigp, \
           tc.tile_pool(name="qT", bufs=1) as qTp, \
           tc.tile_pool(name="qsc", bufs=1) as qsc:
        h128b = cpool.tile([128, 128], BF)
        h128h = cpool.tile([128, 128], F16)
        idb = cpool.tile([128, 128], BF)
        nc.sync.dma_start(h128b[:], h128b_d[:])
        nc.sync.dma_start(h128h[:], h128h_d[:])
        nc.sync.dma_start(idb[:], idb_d[:])

        def scale_finish(tag, cc_out):
            # r = QMAX/s via reciprocal + one newton step
            sg = qsc.tile([1, 1], DT, tag=f"sg{tag}", name=f"sg{tag}")
            nc.sync.dma_start(sg[0:1, :], cc_out[:])
            r0 = qsc.tile([1, 1], DT, tag=f"r0{tag}", name=f"r0{tag}")
            nc.vector.reciprocal(r0[0:1, :], sg[0:1, :])
            mr = qsc.tile([1, 1], DT, tag=f"mr{tag}", name=f"mr{tag}")
            nc.vector.tensor_tensor(mr[0:1, :], sg[0:1, :], r0[0:1, :],
                                    op=A.mult)
            tw = qsc.tile([1, 1], DT, tag=f"tw{tag}", name=f"tw{tag}")
            nc.vector.tensor_scalar(tw[0:1, :], mr[0:1, :], -1.0, 2.0,
                                    op0=A.mult, op1=A.add)
            r1 = qsc.tile([1, 1], DT, tag=f"r1{tag}", name=f"r1{tag}")
            nc.vector.tensor_tensor(r1[0:1, :], r0[0:1, :], tw[0:1, :],
                                    op=A.mult)
            rq = qsc.tile([1, 1], DT, tag=f"rq{tag}", name=f"rq{tag}")
            nc.vector.tensor_scalar_mul(rq[0:1, :], r1[0:1, :], QMAX)
            rb = qsc.tile([128, 1], DT, tag=f"rb{tag}", name=f"rb{tag}")
            nc.gpsimd.partition_broadcast(rb[:, 0:1], rq[0:1, 0:1])
            return sg, rb

        with tc.tile_pool(name="fin", bufs=1) as fin, \
             tc.tile_pool(name="qtmp", bufs=2) as qtmp, \
             tc.tile_pool(name="fps", bufs=4, space="PSUM") as psp:

            # ---------- forward H128 (x) ----------
            xbh = fin.tile([128, 8192], BF, tag="xbh", name="xbh")
            xbl = fin.tile([128, 8192], BF, tag="xbl", name="xbl")
            nc.sync.dma_start(xbh[:], xhh[:])
            nc.sync.dma_start(xbl[:], xhl[:])
            xrB = bigp.tile([128, 8192], DT, tag="big1", name="xrB")
            for j in range(16):
                sl = slice(j * 512, (j + 1) * 512)
                ps = psp.tile([128, 512], DT, tag="fps", name="fpst")
                nc.tensor.matmul(ps[:], h128b[:], xbh[:, sl],
                                 start=True, stop=False)
                nc.tensor.matmul(ps[:], h128b[:], xbl[:, sl],
                                 start=False, stop=True)
                if j % 2 == 0:
                    nc.vector.tensor_copy(xrB[:, sl], ps[:])
                else:
                    nc.scalar.copy(xrB[:, sl], ps[:])
            amx = qsc.tile([128, 1], DT, tag="amx", name="amx")
            nc.vector.tensor_reduce(amx[:], xrB[:],
                                    axis=mybir.AxisListType.X, op=A.max,
                                    apply_absolute_value=True)
            rdx = qsc.tile([128, 1], DT, tag="rdx", name="rdx")
            nc.gpsimd.partition_all_reduce(
                rdx[:], amx[:], channels=128,
                reduce_op=bass_isa.ReduceOp.absmax)
            nc.sync.dma_start(sx_i[:], rdx[0:1, 0:1])
            nc.gpsimd.collective_compute(
                "AllReduce", A.max, replica_groups=rg,
                ins=[sx_i.ap().opt()], outs=[sx_o.ap().opt()])

            # ---------- forward H128 (w) ----------
            wrB = bigp.tile([128, 8192], DT, tag="big2", name="wrB")
            for u in range(4):
                rsl = slice(u * 128, (u + 1) * 128)
                wbh = fin.tile([128, 2048], BF, tag="wbh", name="wbh",
                               bufs=2)
                wbl = fin.tile([128, 2048], BF, tag="wbl", name="wbl",
                               bufs=2)
                nc.sync.dma_start(wbh[:], whh[rsl, :])
                nc.sync.dma_start(wbl[:], whl[rsl, :])
                for j in range(4):
                    sl = slice(j * 512, (j + 1) * 512)
                    ps = psp.tile([128, 512], DT, tag="fps", name="fpsw")
                    nc.tensor.matmul(ps[:], h128b[:], wbh[:, sl],
                                     start=True, stop=False)
                    nc.tensor.matmul(ps[:], h128b[:], wbl[:, sl],
                                     start=False, stop=True)
                    osl = slice(u * 2048 + j * 512, u * 2048 + (j + 1) * 512)
                    if j % 2 == 0:
                        nc.scalar.copy(wrB[:, osl], ps[:])
                    else:
                        nc.vector.tensor_copy(wrB[:, osl], ps[:])
            amw = qsc.tile([128, 1], DT, tag="amw", name="amw")
            nc.vector.tensor_reduce(amw[:], wrB[:],
                                    axis=mybir.AxisListType.X, op=A.max,
                                    apply_absolute_value=True)
            rdw = qsc.tile([128, 1], DT, tag="rdw", name="rdw")
            nc.gpsimd.partition_all_reduce(
                rdw[:], amw[:], channels=128,
                reduce_op=bass_isa.ReduceOp.absmax)
            nc.sync.dma_start(sw_i[:], rdw[0:1, 0:1])
            nc.gpsimd.collective_compute(
                "AllReduce", A.max, replica_groups=rg,
                ins=[sw_i.ap().opt()], outs=[sw_o.ap().opt()])

            def quant(big, rb, noise_ap, qb):
                for ch in range(8):
                    sl = slice(ch * 1024, (ch + 1) * 1024)
                    nz = qtmp.tile([128, 1024], F16, tag="nz", name="nzt")
                    nc.sync.dma_start(nz[:], noise_ap(ch))
                    qi = qtmp.tile([128, 1024], I32, tag="qi", name="qit")
                    nc.vector.scalar_tensor_tensor(
                        qi[:], big[:, sl], rb[:, 0:1], nz[:],
                        op0=A.mult, op1=A.add)
                    if ch % 2 == 0:
                        nc.vector.tensor_copy(qb[:, sl], qi[:])
                    else:
                        nc.scalar.copy(qb[:, sl], qi[:])

            # ---------- x quant + transpose + AllGather ----------
            sgx, rbx = scale_finish("x", sx_o)
            qbx = bigp.tile([128, 8192], BF, tag="qbx", name="qbx")
            quant(xrB, rbx, lambda ch: nk[:, ch * 1024:(ch + 1) * 1024],
                  qbx)
            xqT = [qTp.tile([128, 4096], BF, tag=f"xqT{h}",
                            name=f"xqT{h}") for h in range(2)]
            for h in range(2):
                for quad in range(8):
                    pst = psp.tile([128, 512], BF, tag="pst", name="pstx")
                    for r in range(4):
                        b2 = quad * 4 + r
                        csl = slice(b2 * 256 + h * 128,
                                    b2 * 256 + (h + 1) * 128)
                        nc.tensor.transpose(
                            pst[:, r * 128:(r + 1) * 128], qbx[:, csl],
                            idb[:])
                    osl = slice(quad * 512, (quad + 1) * 512)
                    if quad % 2 == 0:
                        nc.vector.tensor_copy(xqT[h][:, osl], pst[:])
                    else:
                        nc.scalar.copy(xqT[h][:, osl], pst[:])
            nc.sync.dma_start(xqc_a[:], xqT[0][:])
            nc.sync.dma_start(xqc_b[:], xqT[1][:])
            nc.gpsimd.collective_compute(
                "AllGather", A.bypass, replica_groups=rg,
                ins=[xqc_a.ap().opt()], outs=[xqg_a.ap().opt()])
            nc.gpsimd.collective_compute(
                "AllGather", A.bypass, replica_groups=rg,
                ins=[xqc_b.ap().opt()], outs=[xqg_b.ap().opt()])

            # ---------- w quant + transpose (wblk stays in SBUF) ------
            sgw, rbw = scale_finish("w", sw_o)
            qbw = bigp.tile([128, 8192], BF, tag="qbw", name="qbw")
            quant(wrB, rbw,
                  lambda ch: mk[(ch // 2) * 128:(ch // 2 + 1) * 128,
                                (ch % 2) * 1024:(ch % 2 + 1) * 1024],
                  qbw)
            wblk = [qTp.tile([128, 512], BF, tag=f"wb{v}", name=f"wb{v}")
                    for v in range(16)]
            for v in range(16):
                pst = psp.tile([128, 512], BF, tag="pst", name="pstw")
                for u in range(4):
                    csl = slice(u * 2048 + v * 128, u * 2048 + (v + 1) * 128)
                    nc.tensor.transpose(pst[:, u * 128:(u + 1) * 128],
                                        qbw[:, csl], idb[:])
                if v % 2 == 0:
                    nc.scalar.copy(wblk[v][:], pst[:])
                else:
                    nc.vector.tensor_copy(wblk[v][:], pst[:])

            # alpha = sx*sw/(QMAX^2 * 2^24)
            al = qsc.tile([1, 1], DT, tag="al", name="al")
            nc.vector.tensor_tensor(al[0:1, 0:1], sgx[0:1, 0:1],
                                    sgw[0:1, 0:1], op=A.mult)
            nc.vector.tensor_scalar_mul(
                al[0:1, 0:1], al[0:1, 0:1],
                float(1.0 / (QMAX * QMAX * (1 << 24))))
            alb = qsc.tile([128, 1], DT, tag="alb", name="alb")
            nc.gpsimd.partition_broadcast(alb[:, 0:1], al[0:1, 0:1])

        # ---------- GEMM ----------
        yrb = bigp.tile([128, 16384], F16, tag="big1", name="yrb")
        with tc.tile_pool(name="gem", bufs=6) as gem, \
             tc.tile_pool(name="gps", bufs=8, space="PSUM") as gps:
            for g in range(4):
                pss = [gps.tile([128, 512], DT, tag="gp",
                                name=f"gpt{g}_{i}", bufs=8)
                       for i in range(8)]
                for t in list(range(0, 16, 2)) + list(range(1, 16, 2)):
                    k, h = t // 2, t % 2
                    src = xqg_a if h == 0 else xqg_b
                    xt = gem.tile([128, 1024], BF, tag="xt", name="xtt")
                    nc.sync.dma_start(
                        xt[:], src[k * 128:(k + 1) * 128,
                                   g * 1024:(g + 1) * 1024])
                    first = (t == 0)
                    last = (t == 15)
                    for i in range(8):
                        nc.tensor.matmul(
                            pss[i][:], xt[:, i * 128:(i + 1) * 128],
                            wblk[t][:], start=first, stop=last)
                for i in range(8):
                    b2 = g * 8 + i
                    nc.vector.tensor_scalar(
                        yrb[:, b2 * 512:(b2 + 1) * 512], pss[i][:],
                        alb[:, 0:1], None, op0=A.mult)

            # ---------- inverse: fused batch-H128+T, feature-H128 -----
            zT = bigp.tile([128, 16384], F16, tag="big2", name="zT")
            for b2 in range(32):
                ps2 = gps.tile([128, 512], DT, tag="gp", name="ps2t",
                               bufs=8)
                for u in range(4):
                    csl = slice(b2 * 512 + u * 128, b2 * 512 + (u + 1) * 128)
                    nc.tensor.matmul(ps2[:, u * 128:(u + 1) * 128],
                                     yrb[:, csl], h128h[:],
                                     start=True, stop=True)
                osl = slice(b2 * 512, (b2 + 1) * 512)
                if b2 % 2 == 0:
                    nc.scalar.copy(zT[:, osl], ps2[:])
                else:
                    nc.vector.tensor_copy(zT[:, osl], ps2[:])
            for b2 in range(32):
                sl = slice(b2 * 512, (b2 + 1) * 512)
                ps3 = gps.tile([128, 512], DT, tag="gp", name="ps3t",
                               bufs=8)
                nc.tensor.matmul(ps3[:], h128h[:], zT[:, sl],
                                 start=True, stop=True)
                ot = gem.tile([128, 512], F16, tag="ot", name="ott",
                              bufs=4)
                if b2 % 2 == 0:
                    nc.vector.tensor_copy(ot[:], ps3[:])
                else:
                    nc.scalar.copy(ot[:], ps3[:])
                # out[u*128 + p, b2*128 + j] <- ot[p, (u, j)]
                nc.sync.dma_start(
                    out[:, b2 * 128:(b2 + 1) * 128]
                    .rearrange("(u p) j -> p u j", p=128),
                    ot[:].rearrange("p (u j) -> p u j", u=4))
    nc.compile()
    return nc


def make_in_maps(inputs):
    H32 = _sylvester(32)
    x = np.asarray(inputs["inputs"], np.float32)
    w = np.asarray(inputs["kernel"], np.float32)
    nxp = (0.5 - np.asarray(inputs["noise_x"], np.float32))
    nwp = (0.5 - np.asarray(inputs["noise_w"], np.float32))

    # host cross-shard combines (H32 factors)
    xh = np.einsum('st,bti->bsi', H32, x.reshape(128, 32, IN))
    wh = np.einsum('st,itp->isp', H32, w.reshape(IN, 32, 128))
    nx3 = nxp.reshape(128, 32, IN)

    in_maps = []
    for k in range(NCORES):
        xs = np.ascontiguousarray(xh[:, :, k * CS:(k + 1) * CS]) \
               .reshape(128, 8192)
        xs_hi = xs.astype(BF16)
        xs_lo = (xs - xs_hi.astype(np.float32)).astype(BF16)
        nks = np.ascontiguousarray(nx3[:, :, k * CS:(k + 1) * CS]) \
                .reshape(128, 8192).astype(FP16)
        ws = np.ascontiguousarray(wh[:, 4 * k:4 * k + 4, :]
                                  .transpose(1, 2, 0)).reshape(512, IN)
        ws_hi = ws.astype(BF16)
        ws_lo = (ws - ws_hi.astype(np.float32)).astype(BF16)
        mks = np.ascontiguousarray(
            nwp[:, k * FS:(k + 1) * FS].T).astype(FP16)
        in_maps.append({
            "xhh": xs_hi, "xhl": xs_lo, "nk": nks,
            "whh": ws_hi, "whl": ws_lo, "mk": mks,
        })
    return in_maps


def kernel(**inputs):
    from concourse.bass_utils import run_bass_kernel_spmd

    if "nc" not in _cache:
        _cache["nc"] = _build()
    nc = _cache["nc"]

    bias = np.asarray(inputs["bias"], np.float32)
    in_maps = make_in_maps(inputs)

    res = run_bass_kernel_spmd(nc, in_maps, list(range(NCORES)))

    # host unshard: H32 mirror factors over feature-blocks and batch-low
    H32 = _sylvester(32)
    V = np.stack([r["out"].astype(np.float32) for r in res.results])
    V = V.reshape(32, 128, 32, 128)               # [g=(a,u), q, b2, b1]
    V = np.einsum('st,tqbj->sqbj', H32, V)        # H32 over feature blocks
    V = np.einsum('cd,sqdj->sqcj', H32, V)        # H32 over batch-low
    y = V.transpose(3, 2, 0, 1).reshape(B, F)     # [b1, b2, g, q] -> [B, F]
    return (y + bias[None, :]).astype(np.float32)


# revision 11
# speedup vs baseline: 1.6718x; 1.0325x over previous
"""Trainium2 Bass kernel for quantized dense layer with Hadamard rotations.

Math (reference): y = (H2 @ (sq(H2@x) @ sq(w@H1)) @ H1)/4096 + bias,
sq() = per-tensor symmetric int8 stochastic quantization.

Sharding (8 cores), per the data-parallel + per-shard-Hadamard hint:
Sylvester Hadamards factor as Kronecker products; the cross-shard H32
factors are folded into the host-side shard/unshard combines, while
each core applies the per-shard H128 factors on the PE (bf16 hi+lo
split of the fp32 operands, accumulated in one PSUM group — exact to
~2^-18), computes the global quant scales via two 1-scalar AllReduces,
quantizes (stochastic rounding via the fp32->int32 round-to-nearest
cast with host-precomputed 0.5-noise), PE-transposes into GEMM layout,
AllGathers the quantized activations (w arrives feature-sharded so no
AllToAll is needed), runs the int-exact bf16 GEMM, and applies the
inverse per-shard H128 factors (fused transpose matmuls, fp16).

Host: cross-shard H32 combines (pre: batch-low bits of x, feature-high
bits of w; post: the mirror factors on the gathered output), layout
prep, hi/lo dtype split, bias.
"""
import sys
sys.path.insert(0, '/opt/trn_rl_repo')
import numpy as np
import ml_dtypes

B, IN, F = 4096, 2048, 4096
NCORES = 8
CS = IN // NCORES      # 256  per-core IN slice of x
FS = F // NCORES       # 512  per-core feature block of w
QMAX = 127.0
BF16 = ml_dtypes.bfloat16
FP16 = np.float16

_cache = {}


def _sylvester(n):
    h = np.array([[1.0]], dtype=np.float32)
    while h.shape[0] < n:
        h = np.block([[h, h], [h, -h]])
    return h


def _build():
    from concourse import bass, bacc, tile, mybir
    import concourse.bass_isa as bass_isa

    DT = mybir.dt.float32
    BF = mybir.dt.bfloat16
    F16 = mybir.dt.float16
    I32 = mybir.dt.int32
    A = mybir.AluOpType
    npbf = mybir.dt.np(BF)
    npf16 = mybir.dt.np(F16)

    nc = bacc.Bacc("TRN2", target_bir_lowering=False, debug=False,
                   num_devices=NCORES)

    # host-prepped inputs
    xhh = nc.dram_tensor("xhh", [128, 8192], BF, kind="ExternalInput")
    xhl = nc.dram_tensor("xhl", [128, 8192], BF, kind="ExternalInput")
    nk = nc.dram_tensor("nk", [128, 8192], F16, kind="ExternalInput")
    whh = nc.dram_tensor("whh", [512, 2048], BF, kind="ExternalInput")
    whl = nc.dram_tensor("whl", [512, 2048], BF, kind="ExternalInput")
    mk = nc.dram_tensor("mk", [512, 2048], F16, kind="ExternalInput")
    out = nc.dram_tensor("out", [512, 4096], F16, kind="ExternalOutput")

    sx_i = nc.dram_tensor("sx_i", [1, 1], DT)
    sx_o = nc.dram_tensor("sx_o", [1, 1], DT, addr_space="Shared")
    sw_i = nc.dram_tensor("sw_i", [1, 1], DT)
    sw_o = nc.dram_tensor("sw_o", [1, 1], DT, addr_space="Shared")
    # batch-half AllGather payloads: [256 = (h, p), 2048 = batch half]
    xqc_a = nc.dram_tensor("xqc_a", [256, 2048], BF)
    xqc_b = nc.dram_tensor("xqc_b", [256, 2048], BF)
    xqg_a = nc.dram_tensor("xqg_a", [2048, 2048], BF, addr_space="Shared")
    xqg_b = nc.dram_tensor("xqg_b", [2048, 2048], BF, addr_space="Shared")

    h128b_d = nc.inline_tensor(_sylvester(128).astype(npbf), name="h128b")
    h128h_d = nc.inline_tensor(_sylvester(128).astype(npf16), name="h128h")
    idb_d = nc.inline_tensor(np.eye(128, dtype=np.float32).astype(npbf),
                             name="idb")
    rg = [list(range(NCORES))]

    with tile.TileContext(nc) as tc:
      with tc.tile_pool(name="consts", bufs=1) as cpool, \
           tc.tile_pool(name="big", bufs=1) as bigp, \
           tc.tile_pool(name="qT", bufs=1) as qTp, \
           tc.tile_pool(name="qsc", bufs=1) as qsc:
        h128b = cpool.tile([128, 128], BF)
        h128h = cpool.tile([128, 128], F16)
        idb = cpool.tile([128, 128], BF)
        nc.sync.dma_start(h128b[:], h128b_d[:])
        nc.sync.dma_start(h128h[:], h128h_d[:])
        nc.sync.dma_start(idb[:], idb_d[:])

        def scale_finish(tag, cc_out):
            # r = QMAX/s via reciprocal + one newton step:
            # rq = r0*(254 - 127*s*r0) = 127 * r0 * (2 - s*r0)
            sg = qsc.tile([1, 1], DT, tag=f"sg{tag}", name=f"sg{tag}")
            nc.sync.dma_start(sg[0:1, :], cc_out[:])
            r0 = qsc.tile([1, 1], DT, tag=f"r0{tag}", name=f"r0{tag}")
            nc.vector.reciprocal(r0[0:1, :], sg[0:1, :])
            mr = qsc.tile([1, 1], DT, tag=f"mr{tag}", name=f"mr{tag}")
            nc.vector.tensor_tensor(mr[0:1, :], sg[0:1, :], r0[0:1, :],
                                    op=A.mult)
            tw = qsc.tile([1, 1], DT, tag=f"tw{tag}", name=f"tw{tag}")
            nc.vector.tensor_scalar(tw[0:1, :], mr[0:1, :], -QMAX,
                                    2.0 * QMAX, op0=A.mult, op1=A.add)
            rq = qsc.tile([1, 1], DT, tag=f"rq{tag}", name=f"rq{tag}")
            nc.vector.tensor_tensor(rq[0:1, :], r0[0:1, :], tw[0:1, :],
                                    op=A.mult)
            rb = qsc.tile([128, 1], DT, tag=f"rb{tag}", name=f"rb{tag}")
            nc.gpsimd.partition_broadcast(rb[:, 0:1], rq[0:1, 0:1])
            return sg, rb

        with tc.tile_pool(name="fin", bufs=1) as fin, \
             tc.tile_pool(name="qtmp", bufs=2) as qtmp, \
             tc.tile_pool(name="fps", bufs=4, space="PSUM") as psp:

            # ---------- forward H128 (x) ----------
            xrB = bigp.tile([128, 8192], DT, tag="big1", name="xrB")
            amxp = qsc.tile([128, 16], DT, tag="amxp", name="amxp")
            for q in range(4):
                qsl = slice(q * 2048, (q + 1) * 2048)
                xbh = fin.tile([128, 2048], BF, tag="xbh", name="xbh",
                               bufs=2)
                xbl = fin.tile([128, 2048], BF, tag="xbl", name="xbl",
                               bufs=2)
                nc.sync.dma_start(xbh[:], xhh[:, qsl])
                nc.sync.dma_start(xbl[:], xhl[:, qsl])
                for jj in range(4):
                    j = q * 4 + jj
                    sl = slice(j * 512, (j + 1) * 512)
                    lsl = slice(jj * 512, (jj + 1) * 512)
                    ps = psp.tile([128, 512], DT, tag="fps", name="fpst")
                    nc.tensor.matmul(ps[:], h128b[:], xbh[:, lsl],
                                     start=True, stop=False)
                    nc.tensor.matmul(ps[:], h128b[:], xbl[:, lsl],
                                     start=False, stop=True)
                    if j % 2 == 0:
                        nc.vector.tensor_copy(xrB[:, sl], ps[:])
                    else:
                        nc.scalar.copy(xrB[:, sl], ps[:])
                    nc.vector.tensor_reduce(
                        amxp[:, j:j + 1], ps[:], axis=mybir.AxisListType.X,
                        op=A.max, apply_absolute_value=True)
            amx = qsc.tile([128, 1], DT, tag="amx", name="amx")
            nc.vector.tensor_reduce(amx[:], amxp[:],
                                    axis=mybir.AxisListType.X, op=A.max,
                                    apply_absolute_value=True)
            rdx = qsc.tile([128, 1], DT, tag="rdx", name="rdx")
            nc.gpsimd.partition_all_reduce(
                rdx[:], amx[:], channels=128,
                reduce_op=bass_isa.ReduceOp.absmax)
            nc.sync.dma_start(sx_i[:], rdx[0:1, 0:1])
            nc.gpsimd.collective_compute(
                "AllReduce", A.max, replica_groups=rg,
                ins=[sx_i.ap().opt()], outs=[sx_o.ap().opt()])

            # ---------- forward H128 (w) ----------
            wrB = bigp.tile([128, 8192], DT, tag="big2", name="wrB")
            for u in range(4):
                rsl = slice(u * 128, (u + 1) * 128)
                wbh = fin.tile([128, 2048], BF, tag="wbh", name="wbh",
                               bufs=2)
                wbl = fin.tile([128, 2048], BF, tag="wbl", name="wbl",
                               bufs=2)
                nc.sync.dma_start(wbh[:], whh[rsl, :])
                nc.sync.dma_start(wbl[:], whl[rsl, :])
                for j in range(4):
                    sl = slice(j * 512, (j + 1) * 512)
                    ps = psp.tile([128, 512], DT, tag="fps", name="fpsw")
                    nc.tensor.matmul(ps[:], h128b[:], wbh[:, sl],
                                     start=True, stop=False)
                    nc.tensor.matmul(ps[:], h128b[:], wbl[:, sl],
                                     start=False, stop=True)
                    osl = slice(u * 2048 + j * 512, u * 2048 + (j + 1) * 512)
                    if j % 2 == 0:
                        nc.scalar.copy(wrB[:, osl], ps[:])
                    else:
                        nc.vector.tensor_copy(wrB[:, osl], ps[:])
            amw = qsc.tile([128, 1], DT, tag="amw", name="amw")
            nc.vector.tensor_reduce(amw[:], wrB[:],
                                    axis=mybir.AxisListType.X, op=A.max,
                                    apply_absolute_value=True)
            rdw = qsc.tile([128, 1], DT, tag="rdw", name="rdw")
            nc.gpsimd.partition_all_reduce(
                rdw[:], amw[:], channels=128,
                reduce_op=bass_isa.ReduceOp.absmax)
            nc.sync.dma_start(sw_i[:], rdw[0:1, 0:1])
            nc.gpsimd.collective_compute(
                "AllReduce", A.max, replica_groups=rg,
                ins=[sw_i.ap().opt()], outs=[sw_o.ap().opt()])

            def quant(big, rb, noise_ap, qb):
                for ch in range(8):
                    sl = slice(ch * 1024, (ch + 1) * 1024)
                    nz = qtmp.tile([128, 1024], F16, tag="nz", name="nzt")
                    nc.sync.dma_start(nz[:], noise_ap(ch))
                    qi = qtmp.tile([128, 1024], I32, tag="qi", name="qit")
                    nc.vector.scalar_tensor_tensor(
                        qi[:], big[:, sl], rb[:, 0:1], nz[:],
                        op0=A.mult, op1=A.add)
                    if ch % 2 == 0:
                        nc.vector.tensor_copy(qb[:, sl], qi[:])
                    else:
                        nc.scalar.copy(qb[:, sl], qi[:])

            # ---------- x quant + transpose + AllGather (batch halves) --
            sgx, rbx = scale_finish("x", sx_o)
            qbx = bigp.tile([128, 8192], BF, tag="qbx", name="qbx")
            xqT = [qTp.tile([128, 4096], BF, tag=f"xqT{h}",
                            name=f"xqT{h}") for h in range(2)]
            for half, (xqc, xqg) in enumerate(((xqc_a, xqg_a),
                                               (xqc_b, xqg_b))):
                for ch in range(half * 4, half * 4 + 4):
                    sl = slice(ch * 1024, (ch + 1) * 1024)
                    nz = qtmp.tile([128, 1024], F16, tag="nz", name="nzt")
                    nc.sync.dma_start(nz[:], nk[:, sl])
                    qi = qtmp.tile([128, 1024], I32, tag="qi", name="qit")
                    nc.vector.scalar_tensor_tensor(
                        qi[:], xrB[:, sl], rbx[:, 0:1], nz[:],
                        op0=A.mult, op1=A.add)
                    if ch % 2 == 0:
                        nc.vector.tensor_copy(qbx[:, sl], qi[:])
                    else:
                        nc.scalar.copy(qbx[:, sl], qi[:])
                    # transpose this chunk (4 b2-tiles, both c-halves)
                    for h in range(2):
                        pst = psp.tile([128, 512], BF, tag="pst",
                                       name="pstx")
                        for r in range(4):
                            b2 = ch * 4 + r
                            csl = slice(b2 * 256 + h * 128,
                                        b2 * 256 + (h + 1) * 128)
                            nc.tensor.transpose(
                                pst[:, r * 128:(r + 1) * 128], qbx[:, csl],
                                idb[:])
                        osl = slice(ch * 512, (ch + 1) * 512)
                        if h == 0:
                            nc.vector.tensor_copy(xqT[h][:, osl], pst[:])
                        else:
                            nc.scalar.copy(xqT[h][:, osl], pst[:])
                csl = slice(half * 2048, (half + 1) * 2048)
                nc.sync.dma_start(xqc[0:128, :], xqT[0][:, csl])
                nc.sync.dma_start(xqc[128:256, :], xqT[1][:, csl])
                nc.gpsimd.collective_compute(
                    "AllGather", A.bypass, replica_groups=rg,
                    ins=[xqc.ap().opt()], outs=[xqg.ap().opt()])

            # ---------- w quant + transpose (wblk stays in SBUF) ------
            sgw, rbw = scale_finish("w", sw_o)
            qbw = bigp.tile([128, 8192], BF, tag="qbw", name="qbw")
            quant(wrB, rbw,
                  lambda ch: mk[(ch // 2) * 128:(ch // 2 + 1) * 128,
                                (ch % 2) * 1024:(ch % 2 + 1) * 1024],
                  qbw)
            wblk = [qTp.tile([128, 512], BF, tag=f"wb{v}", name=f"wb{v}")
                    for v in range(16)]
            for v in range(16):
                pst = psp.tile([128, 512], BF, tag="pst", name="pstw")
                for u in range(4):
                    csl = slice(u * 2048 + v * 128, u * 2048 + (v + 1) * 128)
                    nc.tensor.transpose(pst[:, u * 128:(u + 1) * 128],
                                        qbw[:, csl], idb[:])
                if v % 2 == 0:
                    nc.scalar.copy(wblk[v][:], pst[:])
                else:
                    nc.vector.tensor_copy(wblk[v][:], pst[:])

            # alpha = sx*sw/(QMAX^2 * 2^24)
            al = qsc.tile([1, 1], DT, tag="al", name="al")
            nc.vector.tensor_tensor(al[0:1, 0:1], sgx[0:1, 0:1],
                                    sgw[0:1, 0:1], op=A.mult)
            nc.vector.tensor_scalar_mul(
                al[0:1, 0:1], al[0:1, 0:1],
                float(1.0 / (QMAX * QMAX * (1 << 24))))
            alb = qsc.tile([128, 1], DT, tag="alb", name="alb")
            nc.gpsimd.partition_broadcast(alb[:, 0:1], al[0:1, 0:1])

        # ---------- GEMM + inverse stage-1 interleaved per group ------
        yrb = bigp.tile([128, 16384], F16, tag="big1", name="yrb")
        zT = bigp.tile([128, 16384], F16, tag="big2", name="zT")
        with tc.tile_pool(name="gem", bufs=16) as gem, \
             tc.tile_pool(name="gps", bufs=8, space="PSUM") as gps:
            for g in range(4):
                src = xqg_a if g < 2 else xqg_b
                co = (g % 2) * 1024
                pss = [gps.tile([128, 512], DT, tag="gp",
                                name=f"gpt{g}_{i}", bufs=8)
                       for i in range(8)]
                for t in range(16):
                    xt = gem.tile([128, 1024], BF, tag="xt", name="xtt")
                    nc.sync.dma_start(
                        xt[:], src[(t // 2) * 256 + (t % 2) * 128:
                                   (t // 2) * 256 + (t % 2) * 128 + 128,
                                   co:co + 1024])
                    for i in range(8):
                        nc.tensor.matmul(
                            pss[i][:], xt[:, i * 128:(i + 1) * 128],
                            wblk[t][:], start=(t == 0), stop=(t == 15))
                for i in range(8):
                    b2 = g * 8 + i
                    nc.vector.tensor_scalar(
                        yrb[:, b2 * 512:(b2 + 1) * 512], pss[i][:],
                        alb[:, 0:1], None, op0=A.mult)
                # inverse stage-1: fused batch-H128 + transpose (per b2)
                for i in range(8):
                    b2 = g * 8 + i
                    ps2 = gps.tile([128, 512], DT, tag="gp",
                                   name=f"ps2t{b2}", bufs=8)
                    for u in range(4):
                        csl = slice(b2 * 512 + u * 128,
                                    b2 * 512 + (u + 1) * 128)
                        nc.tensor.matmul(ps2[:, u * 128:(u + 1) * 128],
                                         yrb[:, csl], h128h[:],
                                         start=True, stop=True)
                    osl = slice(b2 * 512, (b2 + 1) * 512)
                    if b2 % 2 == 0:
                        nc.scalar.copy(zT[:, osl], ps2[:])
                    else:
                        nc.vector.tensor_copy(zT[:, osl], ps2[:])

            # ---------- inverse stage-2: feature-H128 + out DMA -------
            for b2 in range(32):
                sl = slice(b2 * 512, (b2 + 1) * 512)
                ps3 = gps.tile([128, 512], DT, tag="gp", name="ps3t",
                               bufs=8)
                nc.tensor.matmul(ps3[:], h128h[:], zT[:, sl],
                                 start=True, stop=True)
                ot = gem.tile([128, 512], F16, tag="ot", name="ott",
                              bufs=4)
                if b2 % 2 == 0:
                    nc.vector.tensor_copy(ot[:], ps3[:])
                else:
                    nc.scalar.copy(ot[:], ps3[:])
                # out[u*128 + p, b2*128 + j] <- ot[p, (u, j)]
                nc.sync.dma_start(
                    out[:, b2 * 128:(b2 + 1) * 128]
                    .rearrange("(u p) j -> p u j", p=128),
                    ot[:].rearrange("p (u j) -> p u j", u=4))
    nc.compile()
    return nc


def make_in_maps(inputs):
    H32 = _sylvester(32)
    x = np.asarray(inputs["inputs"], np.float32)
    w = np.asarray(inputs["kernel"], np.float32)
    nxp = (0.5 - np.asarray(inputs["noise_x"], np.float32))
    nwp = (0.5 - np.asarray(inputs["noise_w"], np.float32))

    # host cross-shard combines (H32 factors)
    xh = np.einsum('st,bti->bsi', H32, x.reshape(128, 32, IN))
    wh = np.einsum('st,itp->isp', H32, w.reshape(IN, 32, 128))
    nx3 = nxp.reshape(128, 32, IN)

    in_maps = []
    for k in range(NCORES):
        xs = np.ascontiguousarray(xh[:, :, k * CS:(k + 1) * CS]) \
               .reshape(128, 8192)
        xs_hi = xs.astype(BF16)
        xs_lo = (xs - xs_hi.astype(np.float32)).astype(BF16)
        nks = np.ascontiguousarray(nx3[:, :, k * CS:(k + 1) * CS]) \
                .reshape(128, 8192).astype(FP16)
        ws = np.ascontiguousarray(wh[:, 4 * k:4 * k + 4, :]
                                  .transpose(1, 2, 0)).reshape(512, IN)
        ws_hi = ws.astype(BF16)
        ws_lo = (ws - ws_hi.astype(np.float32)).astype(BF16)
        mks = np.ascontiguousarray(
            nwp[:, k * FS:(k + 1) * FS].T).astype(FP16)
        in_maps.append({
            "xhh": xs_hi, "xhl": xs_lo, "nk": nks,
            "whh": ws_hi, "whl": ws_lo, "mk": mks,
        })
    return in_maps


def kernel(**inputs):
    from concourse.bass_utils import run_bass_kernel_spmd

    if "nc" not in _cache:
        _cache["nc"] = _build()
    nc = _cache["nc"]

    bias = np.asarray(inputs["bias"], np.float32)
    in_maps = make_in_maps(inputs)

    res = run_bass_kernel_spmd(nc, in_maps, list(range(NCORES)))

    # host unshard: H32 mirror factors over feature-blocks and batch-low
    H32 = _sylvester(32)
    V = np.stack([r["out"].astype(np.float32) for r in res.results])
    V = V.reshape(32, 128, 32, 128)               # [g=(a,u), q, b2, b1]
    V = np.einsum('st,tqbj->sqbj', H32, V)        # H32 over feature blocks
    V = np.einsum('cd,sqdj->sqcj', H32, V)        # H32 over batch-low
    y = V.transpose(3, 2, 0, 1).reshape(B, F)     # [b1, b2, g, q] -> [B, F]
    return (y + bias[None, :]).astype(np.float32)


# revision 12
# speedup vs baseline: 2.0631x; 1.2340x over previous
"""Trainium2 Bass kernel for quantized dense layer with Hadamard rotations.

Math (reference): y = (H2 @ (sq(H2@x) @ sq(w@H1)) @ H1)/4096 + bias,
sq() = per-tensor symmetric int8 stochastic quantization.

Sharding (8 cores), per the data-parallel + per-shard-Hadamard hint:
Sylvester Hadamards factor as Kronecker products; the cross-shard H32
factors are folded into the host-side shard/unshard combines, while
each core applies the per-shard H128 factors on device.  Forward:
bf16 hi+lo split of the fp32 operands, H128 matmuls accumulated in one
PSUM group (exact to ~2^-18); global quant scales via two 1-scalar
AllReduces; stochastic rounding via the fp32->int32 round-to-nearest
cast with host-precomputed 0.5-noise.  The two inverse per-shard H128
factors are folded into the quantized operands before the GEMM
(H_B (xq wq) H_F = (H_B xq)(wq H_F), stationary-fixed matmuls on the
quant layouts), so the fp16 GEMM output is final up to the alpha
scale.  Quantized activations are exchanged with a 4-way batch-quarter
AllGather pipelined against the GEMM groups; w arrives feature-sharded
so no AllToAll is needed.

Host: cross-shard H32 combines (pre: batch-low bits of x, feature-high
bits of w; post: the mirror factors on the gathered output), layout
prep, hi/lo dtype split, bias.
"""
import sys
sys.path.insert(0, '/opt/trn_rl_repo')
import numpy as np
import ml_dtypes

B, IN, F = 4096, 2048, 4096
NCORES = 8
CS = IN // NCORES      # 256  per-core IN slice of x
FS = F // NCORES       # 512  per-core feature block of w
QMAX = 127.0
BF16 = ml_dtypes.bfloat16
FP16 = np.float16

_cache = {}


def _sylvester(n):
    h = np.array([[1.0]], dtype=np.float32)
    while h.shape[0] < n:
        h = np.block([[h, h], [h, -h]])
    return h


def _build():
    from concourse import bass, bacc, tile, mybir
    import concourse.bass_isa as bass_isa

    DT = mybir.dt.float32
    BF = mybir.dt.bfloat16
    F16 = mybir.dt.float16
    I32 = mybir.dt.int32
    A = mybir.AluOpType
    npbf = mybir.dt.np(BF)
    npf16 = mybir.dt.np(F16)

    nc = bacc.Bacc("TRN2", target_bir_lowering=False, debug=False,
                   num_devices=NCORES)

    # host-prepped inputs
    xhh = nc.dram_tensor("xhh", [128, 8192], BF, kind="ExternalInput")
    xhl = nc.dram_tensor("xhl", [128, 8192], BF, kind="ExternalInput")
    nk = nc.dram_tensor("nk", [128, 8192], F16, kind="ExternalInput")
    whh = nc.dram_tensor("whh", [512, 2048], BF, kind="ExternalInput")
    whl = nc.dram_tensor("whl", [512, 2048], BF, kind="ExternalInput")
    mk = nc.dram_tensor("mk", [512, 2048], F16, kind="ExternalInput")
    out = nc.dram_tensor("out", [512, 4096], F16, kind="ExternalOutput")

    sx_i = nc.dram_tensor("sx_i", [1, 1], DT)
    sx_o = nc.dram_tensor("sx_o", [1, 1], DT, addr_space="Shared")
    sw_i = nc.dram_tensor("sw_i", [1, 1], DT)
    sw_o = nc.dram_tensor("sw_o", [1, 1], DT, addr_space="Shared")
    # batch-quarter AllGather payloads
    xqc = [nc.dram_tensor(f"xqc{q}", [256, 1024], F16) for q in range(4)]
    xqg = [nc.dram_tensor(f"xqg{q}", [2048, 1024], F16,
                          addr_space="Shared") for q in range(4)]

    h128b_d = nc.inline_tensor(_sylvester(128).astype(npbf), name="h128b")
    idh_d = nc.inline_tensor(np.eye(128, dtype=np.float32).astype(npf16),
                             name="idh")
    rg = [list(range(NCORES))]

    with tile.TileContext(nc) as tc:
      with tc.tile_pool(name="consts", bufs=1) as cpool, \
           tc.tile_pool(name="big", bufs=1) as bigp, \
           tc.tile_pool(name="qT", bufs=1) as qTp, \
           tc.tile_pool(name="qsc", bufs=1) as qsc:
        h128b = cpool.tile([128, 128], BF)
        idh = cpool.tile([128, 128], F16)
        nc.sync.dma_start(h128b[:], h128b_d[:])
        nc.sync.dma_start(idh[:], idh_d[:])

        def scale_finish(tag, cc_out):
            # rq = QMAX/s via reciprocal + one fused newton step
            sg = qsc.tile([1, 1], DT, tag=f"sg{tag}", name=f"sg{tag}")
            nc.sync.dma_start(sg[0:1, :], cc_out[:])
            r0 = qsc.tile([1, 1], DT, tag=f"r0{tag}", name=f"r0{tag}")
            nc.vector.reciprocal(r0[0:1, :], sg[0:1, :])
            mr = qsc.tile([1, 1], DT, tag=f"mr{tag}", name=f"mr{tag}")
            nc.vector.tensor_tensor(mr[0:1, :], sg[0:1, :], r0[0:1, :],
                                    op=A.mult)
            tw = qsc.tile([1, 1], DT, tag=f"tw{tag}", name=f"tw{tag}")
            nc.vector.tensor_scalar(tw[0:1, :], mr[0:1, :], -QMAX,
                                    2.0 * QMAX, op0=A.mult, op1=A.add)
            rq = qsc.tile([1, 1], DT, tag=f"rq{tag}", name=f"rq{tag}")
            nc.vector.tensor_tensor(rq[0:1, :], r0[0:1, :], tw[0:1, :],
                                    op=A.mult)
            rb = qsc.tile([128, 1], DT, tag=f"rb{tag}", name=f"rb{tag}")
            nc.gpsimd.partition_broadcast(rb[:, 0:1], rq[0:1, 0:1])
            return sg, rb

        with tc.tile_pool(name="fin", bufs=1) as fin, \
             tc.tile_pool(name="qtmp", bufs=2) as qtmp, \
             tc.tile_pool(name="fps", bufs=4, space="PSUM") as psp:

            # ---------- forward H128 (x): PSUM copies on ACT only ------
            xrB = bigp.tile([128, 8192], DT, tag="big1", name="xrB")
            amxp = qsc.tile([128, 16], DT, tag="amxp", name="amxp")
            for q in range(4):
                qsl = slice(q * 2048, (q + 1) * 2048)
                xbh = fin.tile([128, 2048], BF, tag="xbh", name="xbh",
                               bufs=2)
                xbl = fin.tile([128, 2048], BF, tag="xbl", name="xbl",
                               bufs=2)
                nc.sync.dma_start(xbh[:], xhh[:, qsl])
                nc.sync.dma_start(xbl[:], xhl[:, qsl])
                for jj in range(4):
                    j = q * 4 + jj
                    sl = slice(j * 512, (j + 1) * 512)
                    lsl = slice(jj * 512, (jj + 1) * 512)
                    ps = psp.tile([128, 512], DT, tag="fps", name="fpst")
                    nc.tensor.matmul(ps[:], h128b[:], xbh[:, lsl],
                                     start=True, stop=False)
                    nc.tensor.matmul(ps[:], h128b[:], xbl[:, lsl],
                                     start=False, stop=True)
                    nc.scalar.copy(xrB[:, sl], ps[:])
                    nc.vector.tensor_reduce(
                        amxp[:, j:j + 1], ps[:], axis=mybir.AxisListType.X,
                        op=A.max, apply_absolute_value=True)
            amx = qsc.tile([128, 1], DT, tag="amx", name="amx")
            nc.vector.tensor_reduce(amx[:], amxp[:],
                                    axis=mybir.AxisListType.X, op=A.max,
                                    apply_absolute_value=True)
            rdx = qsc.tile([128, 1], DT, tag="rdx", name="rdx")
            nc.gpsimd.partition_all_reduce(
                rdx[:], amx[:], channels=128,
                reduce_op=bass_isa.ReduceOp.absmax)
            nc.sync.dma_start(sx_i[:], rdx[0:1, 0:1])
            nc.gpsimd.collective_compute(
                "AllReduce", A.max, replica_groups=rg,
                ins=[sx_i.ap().opt()], outs=[sx_o.ap().opt()])

            # ---------- forward H128 (w) ----------
            wrB = bigp.tile([128, 8192], DT, tag="big2", name="wrB")
            amwp = qsc.tile([128, 16], DT, tag="amwp", name="amwp")
            for u in range(4):
                rsl = slice(u * 128, (u + 1) * 128)
                wbh = fin.tile([128, 2048], BF, tag="wbh", name="wbh",
                               bufs=2)
                wbl = fin.tile([128, 2048], BF, tag="wbl", name="wbl",
                               bufs=2)
                nc.sync.dma_start(wbh[:], whh[rsl, :])
                nc.sync.dma_start(wbl[:], whl[rsl, :])
                for j in range(4):
                    sl = slice(j * 512, (j + 1) * 512)
                    ps = psp.tile([128, 512], DT, tag="fps", name="fpsw")
                    nc.tensor.matmul(ps[:], h128b[:], wbh[:, sl],
                                     start=True, stop=False)
                    nc.tensor.matmul(ps[:], h128b[:], wbl[:, sl],
                                     start=False, stop=True)
                    osl = slice(u * 2048 + j * 512, u * 2048 + (j + 1) * 512)
                    nc.scalar.copy(wrB[:, osl], ps[:])
                    nc.vector.tensor_reduce(
                        amwp[:, u * 4 + j:u * 4 + j + 1], ps[:],
                        axis=mybir.AxisListType.X, op=A.max,
                        apply_absolute_value=True)
            amw = qsc.tile([128, 1], DT, tag="amw", name="amw")
            nc.vector.tensor_reduce(amw[:], amwp[:],
                                    axis=mybir.AxisListType.X, op=A.max,
                                    apply_absolute_value=True)
            rdw = qsc.tile([128, 1], DT, tag="rdw", name="rdw")
            nc.gpsimd.partition_all_reduce(
                rdw[:], amw[:], channels=128,
                reduce_op=bass_isa.ReduceOp.absmax)
            nc.sync.dma_start(sw_i[:], rdw[0:1, 0:1])
            nc.gpsimd.collective_compute(
                "AllReduce", A.max, replica_groups=rg,
                ins=[sw_i.ap().opt()], outs=[sw_o.ap().opt()])

            def quant_fold(big, rb, noise_ap, qf, ch):
                """stt+cast+fold one 1024-col chunk of `big` into fp16 qf."""
                sl = slice(ch * 1024, (ch + 1) * 1024)
                nz = qtmp.tile([128, 1024], F16, tag="nz", name="nzt")
                nc.sync.dma_start(nz[:], noise_ap(ch))
                qi = qtmp.tile([128, 1024], I32, tag="qi", name="qit")
                nc.vector.scalar_tensor_tensor(
                    qi[:], big[:, sl], rb[:, 0:1], nz[:],
                    op0=A.mult, op1=A.add)
                qc = qtmp.tile([128, 1024], BF, tag="qc", name="qct")
                if ch % 2 == 0:
                    nc.vector.tensor_copy(qc[:], qi[:])
                else:
                    nc.scalar.copy(qc[:], qi[:])
                for hb in range(2):
                    psf = psp.tile([128, 512], DT, tag="fps", name="psft")
                    nc.tensor.matmul(psf[:], h128b[:],
                                     qc[:, hb * 512:(hb + 1) * 512],
                                     start=True, stop=True)
                    osl = slice(ch * 1024 + hb * 512,
                                ch * 1024 + (hb + 1) * 512)
                    if hb == 0:
                        nc.vector.tensor_copy(qf[:, osl], psf[:])
                    else:
                        nc.scalar.copy(qf[:, osl], psf[:])

            # ---------- x quant+fold+transpose + 4-way AllGather -------
            sgx, rbx = scale_finish("x", sx_o)
            qfx = bigp.tile([128, 8192], F16, tag="qfx", name="qfx")
            xqT = [qTp.tile([128, 4096], F16, tag=f"xqT{h}",
                            name=f"xqT{h}") for h in range(2)]
            for quarter in range(4):
                for ch in range(quarter * 2, quarter * 2 + 2):
                    quant_fold(xrB, rbx,
                               lambda c: nk[:, c * 1024:(c + 1) * 1024],
                               qfx, ch)
                    # transpose chunk: 4 b2-tiles x 2 c-halves
                    for h in range(2):
                        pst = psp.tile([128, 512], F16, tag="pst",
                                       name="pstx")
                        for r in range(4):
                            b2 = ch * 4 + r
                            csl = slice(b2 * 256 + h * 128,
                                        b2 * 256 + (h + 1) * 128)
                            nc.tensor.transpose(
                                pst[:, r * 128:(r + 1) * 128], qfx[:, csl],
                                idh[:])
                        osl = slice(ch * 512, (ch + 1) * 512)
                        if h == 0:
                            nc.vector.tensor_copy(xqT[h][:, osl], pst[:])
                        else:
                            nc.scalar.copy(xqT[h][:, osl], pst[:])
                csl = slice(quarter * 1024, (quarter + 1) * 1024)
                nc.sync.dma_start(xqc[quarter][0:128, :], xqT[0][:, csl])
                nc.sync.dma_start(xqc[quarter][128:256, :], xqT[1][:, csl])
                nc.gpsimd.collective_compute(
                    "AllGather", A.bypass, replica_groups=rg,
                    ins=[xqc[quarter].ap().opt()],
                    outs=[xqg[quarter].ap().opt()])

            # ---------- w quant+fold+transpose (wblk stays in SBUF) ----
            sgw, rbw = scale_finish("w", sw_o)
            qfw = bigp.tile([128, 8192], F16, tag="qfw", name="qfw")
            for ch in range(8):
                quant_fold(wrB, rbw,
                           lambda c: mk[(c // 2) * 128:(c // 2 + 1) * 128,
                                        (c % 2) * 1024:(c % 2 + 1) * 1024],
                           qfw, ch)
            wblk = [qTp.tile([128, 512], F16, tag=f"wb{v}", name=f"wb{v}")
                    for v in range(16)]
            for v in range(16):
                pst = psp.tile([128, 512], F16, tag="pst", name="pstw")
                for u in range(4):
                    csl = slice(u * 2048 + v * 128, u * 2048 + (v + 1) * 128)
                    nc.tensor.transpose(pst[:, u * 128:(u + 1) * 128],
                                        qfw[:, csl], idh[:])
                if v % 2 == 0:
                    nc.scalar.copy(wblk[v][:], pst[:])
                else:
                    nc.vector.tensor_copy(wblk[v][:], pst[:])

            # alpha = sx*sw/(QMAX^2 * 2^24)
            al = qsc.tile([1, 1], DT, tag="al", name="al")
            nc.vector.tensor_tensor(al[0:1, 0:1], sgx[0:1, 0:1],
                                    sgw[0:1, 0:1], op=A.mult)
            nc.vector.tensor_scalar_mul(
                al[0:1, 0:1], al[0:1, 0:1],
                float(1.0 / (QMAX * QMAX * (1 << 24))))
            alb = qsc.tile([128, 1], DT, tag="alb", name="alb")
            nc.gpsimd.partition_broadcast(alb[:, 0:1], al[0:1, 0:1])

        # ---------- GEMM (output is final up to alpha) ----------------
        with tc.tile_pool(name="gem", bufs=8) as gem, \
             tc.tile_pool(name="gps", bufs=8, space="PSUM") as gps:
            for g in range(4):
                psY = [gps.tile([128, 512], DT, tag="gp",
                                name=f"gpt{g}_{j}", bufs=8)
                       for j in range(8)]
                for t in range(16):
                    xt = gem.tile([128, 1024], F16, tag="xt", name="xtt")
                    nc.sync.dma_start(
                        xt[:], xqg[g][(t // 2) * 256 + (t % 2) * 128:
                                      (t // 2) * 256 + (t % 2) * 128 + 128,
                                      :])
                    for s in range(4):
                        for hb in range(2):
                            nc.tensor.matmul(
                                psY[s * 2 + hb][:],
                                wblk[t][:, s * 128:(s + 1) * 128],
                                xt[:, hb * 512:(hb + 1) * 512],
                                start=(t == 0), stop=(t == 15))
                for s in range(4):
                    for hb in range(2):
                        ot = gem.tile([128, 512], F16, tag="ot",
                                      name="ott", bufs=4)
                        nc.vector.tensor_scalar(
                            ot[:], psY[s * 2 + hb][:], alb[:, 0:1], None,
                            op0=A.mult)
                        nc.sync.dma_start(
                            out[s * 128:(s + 1) * 128,
                                g * 1024 + hb * 512:
                                g * 1024 + (hb + 1) * 512], ot[:])
    nc.compile()
    return nc


def make_in_maps(inputs):
    H32 = _sylvester(32)
    x = np.asarray(inputs["inputs"], np.float32)
    w = np.asarray(inputs["kernel"], np.float32)
    nxp = (0.5 - np.asarray(inputs["noise_x"], np.float32))
    nwp = (0.5 - np.asarray(inputs["noise_w"], np.float32))

    # host cross-shard combines (H32 factors)
    xh = np.einsum('st,bti->bsi', H32, x.reshape(128, 32, IN))
    wh = np.einsum('st,itp->isp', H32, w.reshape(IN, 32, 128))
    nx3 = nxp.reshape(128, 32, IN)

    in_maps = []
    for k in range(NCORES):
        xs = np.ascontiguousarray(xh[:, :, k * CS:(k + 1) * CS]) \
               .reshape(128, 8192)
        xs_hi = xs.astype(BF16)
        xs_lo = (xs - xs_hi.astype(np.float32)).astype(BF16)
        nks = np.ascontiguousarray(nx3[:, :, k * CS:(k + 1) * CS]) \
                .reshape(128, 8192).astype(FP16)
        ws = np.ascontiguousarray(wh[:, 4 * k:4 * k + 4, :]
                                  .transpose(1, 2, 0)).reshape(512, IN)
        ws_hi = ws.astype(BF16)
        ws_lo = (ws - ws_hi.astype(np.float32)).astype(BF16)
        mks = np.ascontiguousarray(
            nwp[:, k * FS:(k + 1) * FS].T).astype(FP16)
        in_maps.append({
            "xhh": xs_hi, "xhl": xs_lo, "nk": nks,
            "whh": ws_hi, "whl": ws_lo, "mk": mks,
        })
    return in_maps


def kernel(**inputs):
    from concourse.bass_utils import run_bass_kernel_spmd

    if "nc" not in _cache:
        _cache["nc"] = _build()
    nc = _cache["nc"]

    bias = np.asarray(inputs["bias"], np.float32)
    in_maps = make_in_maps(inputs)

    res = run_bass_kernel_spmd(nc, in_maps, list(range(NCORES)))

    # host unshard: H32 mirror factors over feature-blocks and batch-low
    H32 = _sylvester(32)
    V = np.stack([r["out"].astype(np.float32) for r in res.results])
    V = V.reshape(NCORES, 4, 128, 32, 128)        # [a, u, q, b2, b1]
    V = V.reshape(32, 128, 32, 128)               # [g=(a,u), q, b2, b1]
    V = np.einsum('st,tqbj->sqbj', H32, V)        # H32 over feature blocks
    V = np.einsum('cd,sqdj->sqcj', H32, V)        # H32 over batch-low
    y = V.transpose(3, 2, 0, 1).reshape(B, F)     # [b1, b2, g, q] -> [B, F]
    return (y + bias[None, :]).astype(np.float32)


# revision 13
# speedup vs baseline: 2.0995x; 1.0177x over previous
"""Trainium2 Bass kernel for quantized dense layer with Hadamard rotations.

Math (reference): y = (H2 @ (sq(H2@x) @ sq(w@H1)) @ H1)/4096 + bias,
sq() = per-tensor symmetric int8 stochastic quantization.

Sharding (8 cores), per the data-parallel + per-shard-Hadamard hint:
Sylvester Hadamards factor as Kronecker products; the cross-shard H32
factors are folded into the host-side shard/unshard combines, while
each core applies the per-shard H128 factors on device.  Forward: fp16
operands, H128 PE matmuls with fp32 accumulation; global quant scales
via two 1-scalar AllReduces; stochastic rounding via the fp32->int32
round-to-nearest cast with host-precomputed 0.5-noise.  The two
inverse per-shard H128 factors are folded into the quantized operands
before the GEMM (H_B (xq wq) H_F = (H_B xq)(wq H_F)) as fused
fold+transpose matmuls, so the fp16 GEMM output is final up to the
alpha scale.  Quantized activations are exchanged with a 4-way
batch-quarter AllGather pipelined against the GEMM groups; w arrives
feature-sharded so no AllToAll is needed.

Host: cross-shard H32 combines (pre: batch-low bits of x, feature-high
bits of w; post: the mirror factors on the gathered output), layout
prep, bias.
"""
import sys
sys.path.insert(0, '/opt/trn_rl_repo')
import numpy as np
import ml_dtypes

B, IN, F = 4096, 2048, 4096
NCORES = 8
CS = IN // NCORES      # 256  per-core IN slice of x
FS = F // NCORES       # 512  per-core feature block of w
QMAX = 127.0
BF16 = ml_dtypes.bfloat16
FP16 = np.float16

_cache = {}


def _sylvester(n):
    h = np.array([[1.0]], dtype=np.float32)
    while h.shape[0] < n:
        h = np.block([[h, h], [h, -h]])
    return h


def _build():
    from concourse import bass, bacc, tile, mybir
    import concourse.bass_isa as bass_isa

    DT = mybir.dt.float32
    F16 = mybir.dt.float16
    I32 = mybir.dt.int32
    A = mybir.AluOpType
    npf16 = mybir.dt.np(F16)

    nc = bacc.Bacc("TRN2", target_bir_lowering=False, debug=False,
                   num_devices=NCORES)

    # host-prepped inputs (fp16)
    xh = nc.dram_tensor("xh", [128, 8192], F16, kind="ExternalInput")
    nk = nc.dram_tensor("nk", [128, 8192], F16, kind="ExternalInput")
    wh = nc.dram_tensor("wh", [512, 2048], F16, kind="ExternalInput")
    mk = nc.dram_tensor("mk", [512, 2048], F16, kind="ExternalInput")
    out = nc.dram_tensor("out", [512, 4096], F16, kind="ExternalOutput")

    sx_i = nc.dram_tensor("sx_i", [1, 1], DT)
    sx_o = nc.dram_tensor("sx_o", [1, 1], DT, addr_space="Shared")
    sw_i = nc.dram_tensor("sw_i", [1, 1], DT)
    sw_o = nc.dram_tensor("sw_o", [1, 1], DT, addr_space="Shared")
    # batch-quarter AllGather payloads
    xqc = [nc.dram_tensor(f"xqc{q}", [256, 1024], F16) for q in range(4)]
    xqg = [nc.dram_tensor(f"xqg{q}", [2048, 1024], F16,
                          addr_space="Shared") for q in range(4)]

    h128h_d = nc.inline_tensor(_sylvester(128).astype(npf16), name="h128h")
    rg = [list(range(NCORES))]

    with tile.TileContext(nc) as tc:
      with tc.tile_pool(name="consts", bufs=1) as cpool, \
           tc.tile_pool(name="big", bufs=1) as bigp, \
           tc.tile_pool(name="qT", bufs=1) as qTp, \
           tc.tile_pool(name="qsc", bufs=1) as qsc:
        h128h = cpool.tile([128, 128], F16)
        nc.sync.dma_start(h128h[:], h128h_d[:])

        def scale_finish(tag, cc_out):
            # rq = QMAX / s  (DVE reciprocal is an iterative divide)
            sg = qsc.tile([1, 1], DT, tag=f"sg{tag}", name=f"sg{tag}")
            nc.sync.dma_start(sg[0:1, :], cc_out[:])
            r0 = qsc.tile([1, 1], DT, tag=f"r0{tag}", name=f"r0{tag}")
            nc.vector.reciprocal(r0[0:1, :], sg[0:1, :])
            rq = qsc.tile([1, 1], DT, tag=f"rq{tag}", name=f"rq{tag}")
            nc.vector.tensor_scalar_mul(rq[0:1, :], r0[0:1, :], QMAX)
            rb = qsc.tile([128, 1], DT, tag=f"rb{tag}", name=f"rb{tag}")
            nc.gpsimd.partition_broadcast(rb[:, 0:1], rq[0:1, 0:1])
            return sg, rb

        with tc.tile_pool(name="fin", bufs=1) as fin, \
             tc.tile_pool(name="qtmp", bufs=2) as qtmp, \
             tc.tile_pool(name="fps", bufs=4, space="PSUM") as psp:

            # ---------- forward H128 (x) ----------
            xrB = bigp.tile([128, 8192], DT, tag="big1", name="xrB")
            amxp = qsc.tile([128, 16], DT, tag="amxp", name="amxp")
            for q in range(4):
                qsl = slice(q * 2048, (q + 1) * 2048)
                xb = fin.tile([128, 2048], F16, tag="xb", name="xb", bufs=2)
                nc.sync.dma_start(xb[:], xh[:, qsl])
                for jj in range(4):
                    j = q * 4 + jj
                    sl = slice(j * 512, (j + 1) * 512)
                    lsl = slice(jj * 512, (jj + 1) * 512)
                    ps = psp.tile([128, 512], DT, tag="fps", name="fpst")
                    nc.tensor.matmul(ps[:], h128h[:], xb[:, lsl],
                                     start=True, stop=True)
                    nc.scalar.copy(xrB[:, sl], ps[:])
                    nc.vector.tensor_reduce(
                        amxp[:, j:j + 1], ps[:], axis=mybir.AxisListType.X,
                        op=A.max, apply_absolute_value=True)
            amx = qsc.tile([128, 1], DT, tag="amx", name="amx")
            nc.vector.tensor_reduce(amx[:], amxp[:],
                                    axis=mybir.AxisListType.X, op=A.max,
                                    apply_absolute_value=True)
            rdx = qsc.tile([128, 1], DT, tag="rdx", name="rdx")
            nc.gpsimd.partition_all_reduce(
                rdx[:], amx[:], channels=128,
                reduce_op=bass_isa.ReduceOp.absmax)
            nc.sync.dma_start(sx_i[:], rdx[0:1, 0:1])
            nc.gpsimd.collective_compute(
                "AllReduce", A.max, replica_groups=rg,
                ins=[sx_i.ap().opt()], outs=[sx_o.ap().opt()])

            # ---------- forward H128 (w) ----------
            wrB = bigp.tile([128, 8192], DT, tag="big2", name="wrB")
            amwp = qsc.tile([128, 16], DT, tag="amwp", name="amwp")
            for u in range(4):
                rsl = slice(u * 128, (u + 1) * 128)
                wb = fin.tile([128, 2048], F16, tag="wb", name="wb", bufs=2)
                nc.sync.dma_start(wb[:], wh[rsl, :])
                for j in range(4):
                    sl = slice(j * 512, (j + 1) * 512)
                    ps = psp.tile([128, 512], DT, tag="fps", name="fpsw")
                    nc.tensor.matmul(ps[:], h128h[:], wb[:, sl],
                                     start=True, stop=True)
                    osl = slice(u * 2048 + j * 512, u * 2048 + (j + 1) * 512)
                    nc.scalar.copy(wrB[:, osl], ps[:])
                    nc.vector.tensor_reduce(
                        amwp[:, u * 4 + j:u * 4 + j + 1], ps[:],
                        axis=mybir.AxisListType.X, op=A.max,
                        apply_absolute_value=True)
            amw = qsc.tile([128, 1], DT, tag="amw", name="amw")
            nc.vector.tensor_reduce(amw[:], amwp[:],
                                    axis=mybir.AxisListType.X, op=A.max,
                                    apply_absolute_value=True)
            rdw = qsc.tile([128, 1], DT, tag="rdw", name="rdw")
            nc.gpsimd.partition_all_reduce(
                rdw[:], amw[:], channels=128,
                reduce_op=bass_isa.ReduceOp.absmax)
            nc.sync.dma_start(sw_i[:], rdw[0:1, 0:1])
            nc.gpsimd.collective_compute(
                "AllReduce", A.max, replica_groups=rg,
                ins=[sw_i.ap().opt()], outs=[sw_o.ap().opt()])

            def quant_chunk(big, rb, noise_ap, ch):
                """stt + cast one 1024-col chunk -> fp16 int-valued tile."""
                sl = slice(ch * 1024, (ch + 1) * 1024)
                nz = qtmp.tile([128, 1024], F16, tag="nz", name="nzt")
                nc.sync.dma_start(nz[:], noise_ap(ch))
                qi = qtmp.tile([128, 1024], I32, tag="qi", name="qit")
                nc.vector.scalar_tensor_tensor(
                    qi[:], big[:, sl], rb[:, 0:1], nz[:],
                    op0=A.mult, op1=A.add)
                qc = qtmp.tile([128, 1024], F16, tag="qc", name="qct")
                if ch % 2 == 0:
                    nc.vector.tensor_copy(qc[:], qi[:])
                else:
                    nc.scalar.copy(qc[:], qi[:])
                return qc

            # ---------- x quant + fused fold/transpose + 4-way AG ------
            sgx, rbx = scale_finish("x", sx_o)
            xqT = [qTp.tile([128, 4096], F16, tag=f"xqT{h}",
                            name=f"xqT{h}") for h in range(2)]

            def x_quarter(quarter):
                for ch in range(quarter * 2, quarter * 2 + 2):
                    qc = quant_chunk(
                        xrB, rbx,
                        lambda c: nk[:, c * 1024:(c + 1) * 1024], ch)
                    pstt = [psp.tile([128, 512], DT, tag="pst",
                                     name=f"pstx{h}") for h in range(2)]
                    for p in range(8):
                        r, h = p // 2, p % 2
                        nc.tensor.matmul(
                            pstt[h][:, r * 128:(r + 1) * 128],
                            qc[:, p * 128:(p + 1) * 128], h128h[:],
                            start=True, stop=True)
                    osl = slice(ch * 512, (ch + 1) * 512)
                    nc.vector.tensor_copy(xqT[0][:, osl], pstt[0][:])
                    nc.scalar.copy(xqT[1][:, osl], pstt[1][:])
                csl = slice(quarter * 1024, (quarter + 1) * 1024)
                nc.sync.dma_start(xqc[quarter][0:128, :], xqT[0][:, csl])
                nc.sync.dma_start(xqc[quarter][128:256, :], xqT[1][:, csl])
                nc.gpsimd.collective_compute(
                    "AllGather", A.bypass, replica_groups=rg,
                    ins=[xqc[quarter].ap().opt()],
                    outs=[xqg[quarter].ap().opt()])

            x_quarter(0)

            # ---------- w quant + fused fold/transpose ----------
            sgw, rbw = scale_finish("w", sw_o)
            wblk = qTp.tile([128, 8192], F16, tag="wblk", name="wblk")
            wblk4 = wblk[:].rearrange("p (t s q) -> p t s q", t=16, s=4)
            # process i-half 0 first (v 0..7) so early GEMM k-tiles are ready
            for ch in (0, 2, 4, 6, 1, 3, 5, 7):
                u, hv = ch // 2, ch % 2
                qc = quant_chunk(
                    wrB, rbw,
                    lambda c: mk[(c // 2) * 128:(c // 2 + 1) * 128,
                                 (c % 2) * 1024:(c % 2 + 1) * 1024], ch)
                for q2 in range(2):
                    pst = psp.tile([128, 512], DT, tag="pst", name="pstw")
                    for r in range(4):
                        p = q2 * 4 + r
                        nc.tensor.matmul(
                            pst[:, r * 128:(r + 1) * 128],
                            qc[:, p * 128:(p + 1) * 128], h128h[:],
                            start=True, stop=True)
                    v0 = hv * 8 + q2 * 4
                    dst = wblk4[:, v0:v0 + 4, u, :]
                    src = pst[:].rearrange("p (r q) -> p r q", r=4)
                    if ch % 2 == 0:
                        nc.vector.tensor_copy(dst, src)
                    else:
                        nc.scalar.copy(dst, src)

            for quarter in range(1, 4):
                x_quarter(quarter)

            # alpha = sx*sw/(QMAX^2 * 2^24)
            al = qsc.tile([1, 1], DT, tag="al", name="al")
            nc.vector.tensor_tensor(al[0:1, 0:1], sgx[0:1, 0:1],
                                    sgw[0:1, 0:1], op=A.mult)
            nc.vector.tensor_scalar_mul(
                al[0:1, 0:1], al[0:1, 0:1],
                float(1.0 / (QMAX * QMAX * (1 << 24))))
            alb = qsc.tile([128, 1], DT, tag="alb", name="alb")
            nc.gpsimd.partition_broadcast(alb[:, 0:1], al[0:1, 0:1])

        # ---------- GEMM (output is final up to alpha) ----------------
        with tc.tile_pool(name="gem", bufs=8) as gem, \
             tc.tile_pool(name="gps", bufs=8, space="PSUM") as gps:
            wblk_g = wblk  # keep referenced
            for g in range(4):
                psY = [gps.tile([128, 512], DT, tag="gp",
                                name=f"gpt{g}_{j}", bufs=8)
                       for j in range(8)]
                for t in range(16):
                    xt = gem.tile([128, 1024], F16, tag="xt", name="xtt")
                    nc.sync.dma_start(
                        xt[:], xqg[g][(t // 2) * 256 + (t % 2) * 128:
                                      (t // 2) * 256 + (t % 2) * 128 + 128,
                                      :])
                    for s in range(4):
                        for hb in range(2):
                            nc.tensor.matmul(
                                psY[s * 2 + hb][:],
                                wblk_g[:, t * 512 + s * 128:
                                       t * 512 + (s + 1) * 128],
                                xt[:, hb * 512:(hb + 1) * 512],
                                start=(t == 0), stop=(t == 15))
                for s in range(4):
                    for hb in range(2):
                        ot = gem.tile([128, 512], F16, tag="ot",
                                      name="ott", bufs=4)
                        nc.vector.tensor_scalar(
                            ot[:], psY[s * 2 + hb][:], alb[:, 0:1], None,
                            op0=A.mult)
                        nc.sync.dma_start(
                            out[s * 128:(s + 1) * 128,
                                g * 1024 + hb * 512:
                                g * 1024 + (hb + 1) * 512], ot[:])
    nc.compile()
    return nc


def make_in_maps(inputs):
    H32 = _sylvester(32)
    x = np.asarray(inputs["inputs"], np.float32)
    w = np.asarray(inputs["kernel"], np.float32)
    nxp = (0.5 - np.asarray(inputs["noise_x"], np.float32))
    nwp = (0.5 - np.asarray(inputs["noise_w"], np.float32))

    # host cross-shard combines (H32 factors)
    xhf = np.einsum('st,bti->bsi', H32, x.reshape(128, 32, IN))
    whf = np.einsum('st,itp->isp', H32, w.reshape(IN, 32, 128))
    nx3 = nxp.reshape(128, 32, IN)

    in_maps = []
    for k in range(NCORES):
        xs = np.ascontiguousarray(xhf[:, :, k * CS:(k + 1) * CS]) \
               .reshape(128, 8192).astype(FP16)
        nks = np.ascontiguousarray(nx3[:, :, k * CS:(k + 1) * CS]) \
                .reshape(128, 8192).astype(FP16)
        ws = np.ascontiguousarray(whf[:, 4 * k:4 * k + 4, :]
                                  .transpose(1, 2, 0)) \
               .reshape(512, IN).astype(FP16)
        mks = np.ascontiguousarray(
            nwp[:, k * FS:(k + 1) * FS].T).astype(FP16)
        in_maps.append({"xh": xs, "nk": nks, "wh": ws, "mk": mks})
    return in_maps


def kernel(**inputs):
    from concourse.bass_utils import run_bass_kernel_spmd

    if "nc" not in _cache:
        _cache["nc"] = _build()
    nc = _cache["nc"]

    bias = np.asarray(inputs["bias"], np.float32)
    in_maps = make_in_maps(inputs)

    res = run_bass_kernel_spmd(nc, in_maps, list(range(NCORES)))

    # host unshard: H32 mirror factors over feature-blocks and batch-low
    H32 = _sylvester(32)
    V = np.stack([r["out"].astype(np.float32) for r in res.results])
    V = V.reshape(NCORES, 4, 128, 32, 128)        # [a, u, q, b2, b1]
    V = V.reshape(32, 128, 32, 128)               # [g=(a,u), q, b2, b1]
    V = np.einsum('st,tqbj->sqbj', H32, V)        # H32 over feature blocks
    V = np.einsum('cd,sqdj->sqcj', H32, V)        # H32 over batch-low
    y = V.transpose(3, 2, 0, 1).reshape(B, F)     # [b1, b2, g, q] -> [B, F]
    return (y + bias[None, :]).astype(np.float32)
